# revision 24
# baseline (speedup 1.0000x reference)
"""Multi-head attention (B=4, S=2048, D=1024, H=16, Dh=64) on 8 TRN2 cores.

Sharding: data-parallel over batch (4) x tensor-parallel over heads (2 groups
of 8). Core c handles batch c//2, head-group c%2 (heads g*8..g*8+7 via the
column split of W_q/W_k/W_v and row split of W_o). Each core emits a partial
output projection o^T [1024, 2048]; the host sums the two head-group partials
per batch and transposes back.

All matmuls bf16 (inputs pre-cast on host), fp32 PSUM accumulation:
  Phase A (per head-pair ot): K^T, Q^T, V slices via bf16 matmuls.
  Phase B (per q-chunk qc, head-pair j):
    S^T tiles = K^T.T Q^T   (row-tiled head pairs at (0,0)/(64,0), K=64)
    P = exp(S/8)            (ACT per-ktile, PSUM[128,2,512] -> bf16)
    ctx^T += V.T P^T        (col-tiled pairs at (0,0)/(0,64), M=64)
    sums: DVE bf16 quarter-tree + col-tiled ones matmuls into one PSUM bank
    ctx^T *= 1/sums         (reciprocal_approx_fast + fused PSUM mul)
  Phase C: o^T = Wo_g^T.T @ ctx^T -> DMA out.

PSUM budget (8 banks): psS 2x2 + psC 1 + psR 1 + psAO 2 (projections and
output projection share a pool so phases overlap without false deps).
"""

import numpy as np

import concourse.bacc as bacc
import concourse.mybir as mybir
import concourse.tile as tile

D = 1024  # model dim
S = 2048  # sequence length
O = 512  # per-core projected dim (8 heads x 64)
IT = D // 128  # 8 input-dim tiles
NP = 4  # head pairs per core
QC = S // 512  # 4 q-chunks
KT = S // 128  # 16 k-tiles
VT = S // 128  # 16 s-tiles of V
MT = D // 128  # 8 output m-tiles
SCALE = 0.125  # 1/sqrt(64)

F32 = mybir.dt.float32
BF16 = mybir.dt.bfloat16


def build_kernel():
    nc = bacc.Bacc("TRN2", target_bir_lowering=False, debug=False, num_devices=8)
    xqt = nc.declare_dram_parameter("xqt", [D, S], BF16, isOutput=False)
    xkvt = nc.declare_dram_parameter("xkvt", [D, S], BF16, isOutput=False)
    wqt = nc.declare_dram_parameter("wqt", [D, O], BF16, isOutput=False)
    wkt = nc.declare_dram_parameter("wkt", [D, O], BF16, isOutput=False)
    wvt = nc.declare_dram_parameter("wvt", [D, O], BF16, isOutput=False)
    wot = nc.declare_dram_parameter("wot", [O, D], BF16, isOutput=False)
    ones = nc.declare_dram_parameter("ones", [128, 64], BF16, isOutput=False)
    otp = nc.declare_dram_parameter("otp", [D, S], F32, isOutput=True)

    with tile.TileContext(nc) as tc:
        with (
            tc.tile_pool(name="persist", bufs=1) as persist,
            tc.tile_pool(name="ctxp", bufs=1) as ctxp,
            tc.tile_pool(name="pp", bufs=4) as ppp,
            tc.tile_pool(name="sump", bufs=1) as sump,
            tc.tile_pool(name="rp", bufs=1) as rp,
            tc.tile_pool(name="ots", bufs=2) as ots,
            tc.tile_pool(name="psS", bufs=2, space="PSUM") as psS,
            tc.tile_pool(name="psC", bufs=2, space="PSUM") as psC,
            tc.tile_pool(name="psR", bufs=1, space="PSUM") as psR,
            tc.tile_pool(name="psAO", bufs=1, space="PSUM") as psAO,
        ):
            qt_chunks = [
                [persist.tile([128, 512], BF16, name=f"qtc{j}_{c}") for c in range(QC)]
                for j in range(NP)
            ]
            kt_chunks = [
                [persist.tile([128, 512], BF16, name=f"ktc{j}_{c}") for c in range(QC)]
                for j in range(NP)
            ]
            v_tiles = [persist.tile([128, O], BF16, name=f"vt{t}") for t in range(VT)]
            wo_sb = persist.tile([128, O // 128, D], BF16)
            ones_bf = persist.tile([128, 64], BF16)
            nc.sync.dma_start(out=ones_bf, in_=ones[:, :])

            # ---- Phase A: projections (overlaps phase B via disjoint pools) ----
            with tc.tile_pool(name="wqx", bufs=1) as wqp:
                wq_sb = wqp.tile([128, IT, O], BF16)
                wk_sb = wqp.tile([128, IT, O], BF16)
                wv_sb = wqp.tile([128, IT, O], BF16)
                nc.sync.dma_start(
                    out=wv_sb, in_=wvt[:, :].rearrange("(t p) o -> p t o", p=128)
                )
                nc.sync.dma_start(
                    out=wk_sb, in_=wkt[:, :].rearrange("(t p) o -> p t o", p=128)
                )
                nc.sync.dma_start(
                    out=wq_sb, in_=wqt[:, :].rearrange("(t p) o -> p t o", p=128)
                )
                nc.sync.dma_start(
                    out=wo_sb, in_=wot[:, :].rearrange("(t p) m -> p t m", p=128)
                )
                xqt_r = xqt[:, :].rearrange("(t p) s -> p t s", p=128)
                xkvt_r = xkvt[:, :].rearrange("(t p) s -> p t s", p=128)
                xq_c, xkv_c = [], []
                for c in range(QC):
                    xkv_1 = wqp.tile([128, IT, 512], BF16, name=f"xkv{c}")
                    nc.gpsimd.dma_start(
                        out=xkv_1, in_=xkvt_r[:, :, c * 512 : (c + 1) * 512]
                    )
                    xkv_c.append(xkv_1)
                for c in range(QC):
                    xq_1 = wqp.tile([128, IT, 512], BF16, name=f"xq{c}")
                    nc.gpsimd.dma_start(out=xq_1, in_=xqt_r[:, :, c * 512 : (c + 1) * 512])
                    xq_c.append(xq_1)
                def proj_k(ot):
                    for c in range(QC):
                        ps_k = psAO.tile([128, 512], F32, tag="psAO", name=f"ps_k{ot}_{c}")
                        for it in range(IT):
                            nc.tensor.matmul(
                                ps_k,
                                wk_sb[:, it, ot * 128 : (ot + 1) * 128],
                                xkv_c[c][:, it, :],
                                start=(it == 0),
                                stop=(it == IT - 1),
                            )
                        nc.vector.tensor_copy(kt_chunks[ot][c], ps_k)

                def proj_q(ot):
                    for c in range(QC):
                        ps_q = psAO.tile([128, 512], F32, tag="psAO", name=f"ps_q{ot}_{c}")
                        for it in range(IT):
                            nc.tensor.matmul(
                                ps_q,
                                wq_sb[:, it, ot * 128 : (ot + 1) * 128],
                                xq_c[c][:, it, :],
                                start=(it == 0),
                                stop=(it == IT - 1),
                            )
                        nc.vector.tensor_copy(qt_chunks[ot][c], ps_q)

                def proj_v(vt):
                    ps_v = psAO.tile([128, 512], F32, tag="psAO", name=f"ps_v{vt}")
                    for it in range(IT):
                        nc.tensor.matmul(
                            ps_v,
                            xkv_c[vt // 4][:, it, (vt % 4) * 128 : (vt % 4 + 1) * 128],
                            wv_sb[:, it, :],
                            start=(it == 0),
                            stop=(it == IT - 1),
                        )
                    nc.vector.tensor_copy(v_tiles[vt], ps_v)

                for vt in range(VT):
                    proj_v(vt)
                for ot in range(NP):
                    proj_k(ot)
                    proj_q(ot)

                # ---- Phase B: attention (pair-outer); Phase C: output proj ----
                ctx_tiles = [
                    ctxp.tile([128, NP, 512], BF16, name=f"ctx{c}") for c in range(QC)
                ]
                for j in range(NP):
                    for qc in range(QC):
                        psum_ctx = psC.tile([128, 512], F32, tag="psC")
                        psum_r = psR.tile([128, 512], F32, tag="psR")
                        for qi in range(4):  # quarters of the k range
                            pq = ppp.tile([128, 4, 2, 512], BF16, tag="pp")
                            for ki in range(4):  # per-ktile ACT batches
                                kt = qi * 4 + ki
                                ps_s = psS.tile([128, 2, 512], F32, tag="psS")
                                with tc.high_priority():
                                    nc.tensor.matmul(
                                        ps_s[:, 0, :],
                                        kt_chunks[j][kt // 4][0:64, (kt % 4) * 128 : (kt % 4 + 1) * 128],
                                        qt_chunks[j][qc][0:64, :],
                                        start=True,
                                        stop=True,
                                        tile_position=(0, 0),
                                    )
                                    nc.tensor.matmul(
                                        ps_s[:, 1, :],
                                        kt_chunks[j][kt // 4][64:128, (kt % 4) * 128 : (kt % 4 + 1) * 128],
                                        qt_chunks[j][qc][64:128, :],
                                        start=True,
                                        stop=True,
                                        tile_position=(64, 0),
                                    )
                                    nc.scalar.activation(
                                        out=pq[:, ki, :, :],
                                        in_=ps_s[:, :, :],
                                        func=mybir.ActivationFunctionType.Exp,
                                        scale=SCALE,
                                    )
                            # PV + sums for this quarter
                            with tc.high_priority():
                                for ki in range(4):
                                    kt = qi * 4 + ki
                                    first = kt == 0
                                    last = kt == KT - 1
                                    nc.tensor.matmul(
                                        psum_ctx[0:64, :],
                                        v_tiles[kt][:, j * 128 : j * 128 + 64],
                                        pq[:, ki, 0, :],
                                        start=first,
                                        stop=last,
                                        tile_position=(0, 0),
                                    )
                                    nc.tensor.matmul(
                                        psum_ctx[64:128, :],
                                        v_tiles[kt][:, j * 128 + 64 : (j + 1) * 128],
                                        pq[:, ki, 1, :],
                                        start=first,
                                        stop=last,
                                        tile_position=(0, 64),
                                    )
                                tq = sump.tile([128, 2, 2, 512], BF16, tag="tq")
                                s1 = sump.tile([128, 2, 512], BF16, tag="s1")
                                with nc.allow_low_precision(reason="softmax sum partials"):
                                    nc.vector.tensor_add(
                                        tq, pq[:, 0:2, :, :], pq[:, 2:4, :, :]
                                    )
                                    nc.vector.tensor_add(
                                        s1, tq[:, 0, :, :], tq[:, 1, :, :]
                                    )
                                nc.tensor.matmul(
                                    psum_r[0:64, :],
                                    ones_bf,
                                    s1[:, 0, :],
                                    start=(qi == 0),
                                    stop=(qi == 3),
                                    tile_position=(0, 0),
                                )
                                nc.tensor.matmul(
                                    psum_r[64:128, :],
                                    ones_bf,
                                    s1[:, 1, :],
                                    start=(qi == 0),
                                    stop=(qi == 3),
                                    tile_position=(0, 64),
                                )
                        # normalize
                        with tc.high_priority():
                            r_tile = rp.tile([128, 512], F32, tag="r")
                            nc.vector.reciprocal_approx_fast(out=r_tile, in_=psum_r)
                            with nc.allow_low_precision(reason="bf16 ctx for PE"):
                                nc.vector.tensor_mul(
                                    ctx_tiles[qc][:, j, :], psum_ctx, r_tile
                                )

                # Phase C: output projection per q-chunk
                for qc in range(QC):
                    qsl = slice(qc * 512, (qc + 1) * 512)
                    for mt in range(MT):
                        ps_o = psAO.tile([128, 512], F32, tag="psAO", name=f"ps_o{qc}_{mt}")
                        for jt in range(NP):
                            nc.tensor.matmul(
                                ps_o,
                                wo_sb[:, jt, mt * 128 : (mt + 1) * 128],
                                ctx_tiles[qc][:, jt, :],
                                start=(jt == 0),
                                stop=(jt == NP - 1),
                            )
                        ot_sb = ots.tile([128, 512], F32, tag="ot")
                        nc.vector.tensor_copy(ot_sb, ps_o)
                        nc.sync.dma_start(
                            out=otp[mt * 128 : (mt + 1) * 128, qsl], in_=ot_sb
                        )
    nc.compile()
    return nc


def make_in_maps(query_input, kv_input, W_q, W_k, W_v, W_o):
    import ml_dtypes

    bf16 = ml_dtypes.bfloat16
    q = np.asarray(query_input, dtype=np.float32).astype(bf16)
    kv = np.asarray(kv_input, dtype=np.float32).astype(bf16)
    W_q = np.asarray(W_q, dtype=np.float32).astype(bf16)
    W_k = np.asarray(W_k, dtype=np.float32).astype(bf16)
    W_v = np.asarray(W_v, dtype=np.float32).astype(bf16)
    W_o = np.asarray(W_o, dtype=np.float32).astype(bf16)
    ones = np.ones((128, 64), dtype=bf16)
    in_maps = []
    for c in range(8):
        b, g = c // 2, c % 2
        sl = slice(g * O, (g + 1) * O)
        in_maps.append(
            {
                "xqt": np.ascontiguousarray(q[b].T),
                "xkvt": np.ascontiguousarray(kv[b].T),
                "wqt": np.ascontiguousarray(W_q[sl, :].T),
                "wkt": np.ascontiguousarray(W_k[sl, :].T),
                "wvt": np.ascontiguousarray(W_v[sl, :].T),
                "wot": np.ascontiguousarray(W_o[:, sl].T),
                "ones": ones,
            }
        )
    return in_maps


def assemble_output(results):
    out = np.empty((4, S, D), dtype=np.float32)
    for b in range(4):
        partial = results[2 * b]["otp"] + results[2 * b + 1]["otp"]  # [D, S]
        out[b] = partial.T
    return out


_NC_CACHE = None


def kernel(**inputs) -> np.ndarray:
    global _NC_CACHE
    from concourse.bass_utils import run_bass_kernel_spmd

    if _NC_CACHE is None:
        _NC_CACHE = build_kernel()
    in_maps = make_in_maps(
        inputs["query_input"],
        inputs["kv_input"],
        inputs["W_q"],
        inputs["W_k"],
        inputs["W_v"],
        inputs["W_o"],
    )
    res = run_bass_kernel_spmd(_NC_CACHE, in_maps, list(range(8)))
    return assemble_output(res.results)


# revision 26
# speedup vs baseline: 1.1078x; 1.1078x over previous
"""Multi-head attention (B=4, S=2048, D=1024, H=16, Dh=64) on 8 TRN2 cores.

Sharding: data-parallel over batch (4) x tensor-parallel over heads (2 groups
of 8). Core c handles batch c//2, head-group c%2 (heads g*8..g*8+7 via the
column split of W_q/W_k/W_v and row split of W_o). Each core emits a partial
output projection o^T [1024, 2048]; the host sums the two head-group partials
per batch and transposes back.

All matmuls bf16 (inputs pre-cast on host), fp32 PSUM accumulation:
  Phase A (per head-pair ot): K^T, Q^T, V slices via bf16 matmuls.
  Phase B (per q-chunk qc, head-pair j):
    S^T tiles = K^T.T Q^T   (row-tiled head pairs at (0,0)/(64,0), K=64)
    P = exp(S/8)            (ACT per-ktile, PSUM[128,2,512] -> bf16)
    ctx^T += V.T P^T        (col-tiled pairs at (0,0)/(0,64), M=64)
    sums: DVE bf16 quarter-tree + col-tiled ones matmuls into one PSUM bank
    ctx^T *= 1/sums         (reciprocal_approx_fast + fused PSUM mul)
  Phase C: o^T = Wo_g^T.T @ ctx^T -> DMA out.

PSUM budget (8 banks): psS 2x2 + psC 1 + psR 1 + psAO 2 (projections and
output projection share a pool so phases overlap without false deps).
"""

import numpy as np

import concourse.bacc as bacc
import concourse.mybir as mybir
import concourse.tile as tile

D = 1024  # model dim
S = 2048  # sequence length
O = 512  # per-core projected dim (8 heads x 64)
IT = D // 128  # 8 input-dim tiles
NP = 4  # head pairs per core
QC = S // 512  # 4 q-chunks
KT = S // 128  # 16 k-tiles
VT = S // 128  # 16 s-tiles of V
MT = D // 128  # 8 output m-tiles
SCALE = 0.125  # 1/sqrt(64)

F32 = mybir.dt.float32
BF16 = mybir.dt.bfloat16


def build_kernel():
    nc = bacc.Bacc("TRN2", target_bir_lowering=False, debug=False, num_devices=8)
    xqt = nc.declare_dram_parameter("xqt", [QC, 128, IT, 512], BF16, isOutput=False)
    xkvt = nc.declare_dram_parameter("xkvt", [QC, 128, IT, 512], BF16, isOutput=False)
    wqt = nc.declare_dram_parameter("wqt", [128, IT, O], BF16, isOutput=False)
    wkt = nc.declare_dram_parameter("wkt", [128, IT, O], BF16, isOutput=False)
    wvt = nc.declare_dram_parameter("wvt", [128, IT, O], BF16, isOutput=False)
    wot = nc.declare_dram_parameter("wot", [128, O // 128, D], BF16, isOutput=False)
    ones = nc.declare_dram_parameter("ones", [128, 64], BF16, isOutput=False)
    otp = nc.declare_dram_parameter("otp", [D, S], F32, isOutput=True)

    with tile.TileContext(nc) as tc:
        with (
            tc.tile_pool(name="persist", bufs=1) as persist,
            tc.tile_pool(name="ctxp", bufs=1) as ctxp,
            tc.tile_pool(name="pp", bufs=3) as ppp,
            tc.tile_pool(name="sump", bufs=2) as sump,
            tc.tile_pool(name="rp", bufs=2) as rp,
            tc.tile_pool(name="ots", bufs=2) as ots,
            tc.tile_pool(name="psS", bufs=2, space="PSUM") as psS,
            tc.tile_pool(name="psC", bufs=1, space="PSUM") as psC,
            tc.tile_pool(name="psR", bufs=1, space="PSUM") as psR,
            tc.tile_pool(name="psAO", bufs=2, space="PSUM") as psAO,
        ):
            qt_chunks = [
                [persist.tile([128, 512], BF16, name=f"qtc{j}_{c}") for c in range(QC)]
                for j in range(NP)
            ]
            kt_chunks = [
                [persist.tile([128, 512], BF16, name=f"ktc{j}_{c}") for c in range(QC)]
                for j in range(NP)
            ]
            v_tiles = [persist.tile([128, O], BF16, name=f"vt{t}") for t in range(VT)]
            wo_sb = persist.tile([128, O // 128, D], BF16)
            ones_bf = persist.tile([128, 64], BF16)
            nc.sync.dma_start(out=ones_bf, in_=ones[:, :])

            # ---- Phase A: projections (overlaps phase B via disjoint pools) ----
            with tc.tile_pool(name="wqx", bufs=1) as wqp:
                wq_sb = wqp.tile([128, IT, O], BF16)
                wk_sb = wqp.tile([128, IT, O], BF16)
                wv_sb = wqp.tile([128, IT, O], BF16)
                nc.sync.dma_start(
                    out=wv_sb, in_=wvt[:, :, :]
                )
                nc.sync.dma_start(
                    out=wk_sb, in_=wkt[:, :, :]
                )
                nc.sync.dma_start(
                    out=wq_sb, in_=wqt[:, :, :]
                )
                nc.sync.dma_start(
                    out=wo_sb, in_=wot[:, :, :]
                )
                xq_c, xkv_c = [], []
                for c in range(QC):
                    xkv_1 = wqp.tile([128, IT, 512], BF16, name=f"xkv{c}")
                    nc.gpsimd.dma_start(out=xkv_1, in_=xkvt[c, :, :, :])
                    xkv_c.append(xkv_1)
                for c in range(QC):
                    xq_1 = wqp.tile([128, IT, 512], BF16, name=f"xq{c}")
                    nc.gpsimd.dma_start(out=xq_1, in_=xqt[c, :, :, :])
                    xq_c.append(xq_1)
                def proj_k(ot):
                    for c in range(QC):
                        ps_k = psAO.tile([128, 512], F32, tag="psAO", name=f"ps_k{ot}_{c}")
                        for it in range(IT):
                            nc.tensor.matmul(
                                ps_k,
                                wk_sb[:, it, ot * 128 : (ot + 1) * 128],
                                xkv_c[c][:, it, :],
                                start=(it == 0),
                                stop=(it == IT - 1),
                            )
                        nc.vector.tensor_copy(kt_chunks[ot][c], ps_k)

                def proj_q(ot):
                    for c in range(QC):
                        ps_q = psAO.tile([128, 512], F32, tag="psAO", name=f"ps_q{ot}_{c}")
                        for it in range(IT):
                            nc.tensor.matmul(
                                ps_q,
                                wq_sb[:, it, ot * 128 : (ot + 1) * 128],
                                xq_c[c][:, it, :],
                                start=(it == 0),
                                stop=(it == IT - 1),
                            )
                        nc.vector.tensor_copy(qt_chunks[ot][c], ps_q)

                def proj_v(vt):
                    ps_v = psAO.tile([128, 512], F32, tag="psAO", name=f"ps_v{vt}")
                    for it in range(IT):
                        nc.tensor.matmul(
                            ps_v,
                            xkv_c[vt // 4][:, it, (vt % 4) * 128 : (vt % 4 + 1) * 128],
                            wv_sb[:, it, :],
                            start=(it == 0),
                            stop=(it == IT - 1),
                        )
                    nc.vector.tensor_copy(v_tiles[vt], ps_v)

                for vt in range(VT):
                    proj_v(vt)
                for ot in range(NP):
                    proj_k(ot)
                    proj_q(ot)

                # ---- Phase B: attention (pair-outer); Phase C: output proj ----
                ctx_tiles = [
                    ctxp.tile([128, NP, 512], BF16, name=f"ctx{c}") for c in range(QC)
                ]
                for j in range(NP):
                    for qc in range(QC):
                        psum_ctx = psC.tile([128, 512], F32, tag="psC")
                        psum_r = psR.tile([128, 512], F32, tag="psR")
                        for qi in range(4):  # quarters of the k range
                            pq = ppp.tile([128, 4, 2, 512], BF16, tag="pp")
                            for ki in range(4):  # per-ktile ACT batches
                                kt = qi * 4 + ki
                                ps_s = psS.tile([128, 2, 512], F32, tag="psS")
                                with tc.high_priority():
                                    nc.tensor.matmul(
                                        ps_s[:, 0, :],
                                        kt_chunks[j][kt // 4][0:64, (kt % 4) * 128 : (kt % 4 + 1) * 128],
                                        qt_chunks[j][qc][0:64, :],
                                        start=True,
                                        stop=True,
                                        tile_position=(0, 0),
                                    )
                                    nc.tensor.matmul(
                                        ps_s[:, 1, :],
                                        kt_chunks[j][kt // 4][64:128, (kt % 4) * 128 : (kt % 4 + 1) * 128],
                                        qt_chunks[j][qc][64:128, :],
                                        start=True,
                                        stop=True,
                                        tile_position=(64, 0),
                                    )
                                    nc.scalar.activation(
                                        out=pq[:, ki, :, :],
                                        in_=ps_s[:, :, :],
                                        func=mybir.ActivationFunctionType.Exp,
                                        scale=SCALE,
                                    )
                            # PV + sums for this quarter
                            with tc.high_priority():
                                for ki in range(4):
                                    kt = qi * 4 + ki
                                    first = kt == 0
                                    last = kt == KT - 1
                                    nc.tensor.matmul(
                                        psum_ctx[0:64, :],
                                        v_tiles[kt][:, j * 128 : j * 128 + 64],
                                        pq[:, ki, 0, :],
                                        start=first,
                                        stop=last,
                                        tile_position=(0, 0),
                                    )
                                    nc.tensor.matmul(
                                        psum_ctx[64:128, :],
                                        v_tiles[kt][:, j * 128 + 64 : (j + 1) * 128],
                                        pq[:, ki, 1, :],
                                        start=first,
                                        stop=last,
                                        tile_position=(0, 64),
                                    )
                                tq = sump.tile([128, 2, 2, 512], BF16, tag="tq")
                                s1 = sump.tile([128, 2, 512], BF16, tag="s1")
                                with nc.allow_low_precision(reason="softmax sum partials"):
                                    nc.vector.tensor_add(
                                        tq, pq[:, 0:2, :, :], pq[:, 2:4, :, :]
                                    )
                                    nc.vector.tensor_add(
                                        s1, tq[:, 0, :, :], tq[:, 1, :, :]
                                    )
                                nc.tensor.matmul(
                                    psum_r[0:64, :],
                                    ones_bf,
                                    s1[:, 0, :],
                                    start=(qi == 0),
                                    stop=(qi == 3),
                                    tile_position=(0, 0),
                                )
                                nc.tensor.matmul(
                                    psum_r[64:128, :],
                                    ones_bf,
                                    s1[:, 1, :],
                                    start=(qi == 0),
                                    stop=(qi == 3),
                                    tile_position=(0, 64),
                                )
                        # normalize
                        with tc.high_priority():
                            r_tile = rp.tile([128, 512], F32, tag="r")
                            nc.vector.reciprocal_approx_fast(out=r_tile, in_=psum_r)
                            with nc.allow_low_precision(reason="bf16 ctx for PE"):
                                nc.vector.tensor_mul(
                                    ctx_tiles[qc][:, j, :], psum_ctx, r_tile
                                )

                # Phase C: output projection per q-chunk
                for qc in range(QC):
                    qsl = slice(qc * 512, (qc + 1) * 512)
                    for mt in range(MT):
                        ps_o = psAO.tile([128, 512], F32, tag="psAO", name=f"ps_o{qc}_{mt}")
                        for jt in range(NP):
                            nc.tensor.matmul(
                                ps_o,
                                wo_sb[:, jt, mt * 128 : (mt + 1) * 128],
                                ctx_tiles[qc][:, jt, :],
                                start=(jt == 0),
                                stop=(jt == NP - 1),
                            )
                        ot_sb = ots.tile([128, 512], F32, tag="ot")
                        nc.vector.tensor_copy(ot_sb, ps_o)
                        nc.sync.dma_start(
                            out=otp[mt * 128 : (mt + 1) * 128, qsl], in_=ot_sb
                        )
    nc.compile()
    return nc


def make_in_maps(query_input, kv_input, W_q, W_k, W_v, W_o):
    import ml_dtypes

    bf16 = ml_dtypes.bfloat16
    q = np.asarray(query_input, dtype=np.float32).astype(bf16)
    kv = np.asarray(kv_input, dtype=np.float32).astype(bf16)
    W_q = np.asarray(W_q, dtype=np.float32).astype(bf16)
    W_k = np.asarray(W_k, dtype=np.float32).astype(bf16)
    W_v = np.asarray(W_v, dtype=np.float32).astype(bf16)
    W_o = np.asarray(W_o, dtype=np.float32).astype(bf16)
    ones = np.ones((128, 64), dtype=bf16)
    def tile_x(xt):  # [D, S] -> [QC, 128, IT, 512]
        return np.ascontiguousarray(
            xt.reshape(IT, 128, QC, 512).transpose(2, 1, 0, 3)
        )

    def tile_w(wt):  # [D, O] -> [128, IT, O]
        return np.ascontiguousarray(wt.reshape(IT, 128, O).transpose(1, 0, 2))

    in_maps = []
    for c in range(8):
        b, g = c // 2, c % 2
        sl = slice(g * O, (g + 1) * O)
        in_maps.append(
            {
                "xqt": tile_x(q[b].T),
                "xkvt": tile_x(kv[b].T),
                "wqt": tile_w(W_q[sl, :].T),
                "wkt": tile_w(W_k[sl, :].T),
                "wvt": tile_w(W_v[sl, :].T),
                "wot": np.ascontiguousarray(
                    W_o[:, sl].T.reshape(O // 128, 128, D).transpose(1, 0, 2)
                ),
                "ones": ones,
            }
        )
    return in_maps


def assemble_output(results):
    out = np.empty((4, S, D), dtype=np.float32)
    for b in range(4):
        partial = results[2 * b]["otp"] + results[2 * b + 1]["otp"]  # [D, S]
        out[b] = partial.T
    return out


_NC_CACHE = None


def kernel(**inputs) -> np.ndarray:
    global _NC_CACHE
    from concourse.bass_utils import run_bass_kernel_spmd

    if _NC_CACHE is None:
        _NC_CACHE = build_kernel()
    in_maps = make_in_maps(
        inputs["query_input"],
        inputs["kv_input"],
        inputs["W_q"],
        inputs["W_k"],
        inputs["W_v"],
        inputs["W_o"],
    )
    res = run_bass_kernel_spmd(_NC_CACHE, in_maps, list(range(8)))
    return assemble_output(res.results)


# revision 27
# speedup vs baseline: 1.1645x; 1.0511x over previous
"""Multi-head attention (B=4, S=2048, D=1024, H=16, Dh=64) on 8 TRN2 cores.

Sharding: data-parallel over batch (4) x tensor-parallel over heads (2 groups
of 8). Core c handles batch c//2, head-group c%2 (heads g*8..g*8+7 via the
column split of W_q/W_k/W_v and row split of W_o). Each core emits a partial
output projection o^T [1024, 2048]; the host sums the two head-group partials
per batch and transposes back.

All matmuls bf16 (inputs pre-cast on host), fp32 PSUM accumulation:
  Phase A (per head-pair ot): K^T, Q^T, V slices via bf16 matmuls.
  Phase B (per q-chunk qc, head-pair j):
    S^T tiles = K^T.T Q^T   (row-tiled head pairs at (0,0)/(64,0), K=64)
    P = exp(S/8)            (ACT per-ktile, PSUM[128,2,512] -> bf16)
    ctx^T += V.T P^T        (col-tiled pairs at (0,0)/(0,64), M=64)
    sums: DVE bf16 quarter-tree + col-tiled ones matmuls into one PSUM bank
    ctx^T *= 1/sums         (reciprocal_approx_fast + fused PSUM mul)
  Phase C: o^T = Wo_g^T.T @ ctx^T -> DMA out.

PSUM budget (8 banks): psS 2x2 + psC 1 + psR 1 + psAO 2 (projections and
output projection share a pool so phases overlap without false deps).
"""

import numpy as np

import concourse.bacc as bacc
import concourse.mybir as mybir
import concourse.tile as tile

D = 1024  # model dim
S = 2048  # sequence length
O = 512  # per-core projected dim (8 heads x 64)
IT = D // 128  # 8 input-dim tiles
NP = 4  # head pairs per core
QC = S // 512  # 4 q-chunks
KT = S // 128  # 16 k-tiles
VT = S // 128  # 16 s-tiles of V
MT = D // 128  # 8 output m-tiles
SCALE = 0.125  # 1/sqrt(64)

F32 = mybir.dt.float32
BF16 = mybir.dt.bfloat16


def build_kernel():
    nc = bacc.Bacc("TRN2", target_bir_lowering=False, debug=False, num_devices=8)
    xqt = nc.declare_dram_parameter("xqt", [QC, 128, IT, 512], BF16, isOutput=False)
    xkvt = nc.declare_dram_parameter("xkvt", [QC, 128, IT, 512], BF16, isOutput=False)
    wqt = nc.declare_dram_parameter("wqt", [128, IT, O], BF16, isOutput=False)
    wkt = nc.declare_dram_parameter("wkt", [128, IT, O], BF16, isOutput=False)
    wvt = nc.declare_dram_parameter("wvt", [128, IT, O], BF16, isOutput=False)
    wot = nc.declare_dram_parameter("wot", [128, O // 128, D], BF16, isOutput=False)
    ones = nc.declare_dram_parameter("ones", [128, 64], BF16, isOutput=False)
    otp = nc.declare_dram_parameter("otp", [D, S], F32, isOutput=True)

    with tile.TileContext(nc) as tc:
        with (
            tc.tile_pool(name="persist", bufs=1) as persist,
            tc.tile_pool(name="ctxp", bufs=1) as ctxp,
            tc.tile_pool(name="pp", bufs=3) as ppp,
            tc.tile_pool(name="sump", bufs=2) as sump,
            tc.tile_pool(name="rp", bufs=2) as rp,
            tc.tile_pool(name="ots", bufs=2) as ots,
            tc.tile_pool(name="psS", bufs=2, space="PSUM") as psS,
            tc.tile_pool(name="psC", bufs=1, space="PSUM") as psC,
            tc.tile_pool(name="psR", bufs=1, space="PSUM") as psR,
            tc.tile_pool(name="psAO", bufs=2, space="PSUM") as psAO,
        ):
            qt_chunks = [
                [persist.tile([128, 512], BF16, name=f"qtc{j}_{c}") for c in range(QC)]
                for j in range(NP)
            ]
            kt_chunks = [
                [persist.tile([128, 512], BF16, name=f"ktc{j}_{c}") for c in range(QC)]
                for j in range(NP)
            ]
            v_tiles = [persist.tile([128, O], BF16, name=f"vt{t}") for t in range(VT)]
            wo_sb = persist.tile([128, O // 128, D], BF16)
            ones_bf = persist.tile([128, 64], BF16)
            nc.sync.dma_start(out=ones_bf, in_=ones[:, :])

            # ---- Phase A: projections (overlaps phase B via disjoint pools) ----
            with tc.tile_pool(name="wqx", bufs=1) as wqp:
                wq_sb = wqp.tile([128, IT, O], BF16)
                wk_sb = wqp.tile([128, IT, O], BF16)
                wv_sb = wqp.tile([128, IT, O], BF16)
                nc.sync.dma_start(
                    out=wv_sb, in_=wvt[:, :, :]
                )
                nc.sync.dma_start(
                    out=wk_sb, in_=wkt[:, :, :]
                )
                nc.sync.dma_start(
                    out=wq_sb, in_=wqt[:, :, :]
                )
                nc.sync.dma_start(
                    out=wo_sb, in_=wot[:, :, :]
                )
                xq_c, xkv_c = [], []
                for c in range(QC):
                    xkv_1 = wqp.tile([128, IT, 512], BF16, name=f"xkv{c}")
                    nc.gpsimd.dma_start(out=xkv_1, in_=xkvt[c, :, :, :])
                    xkv_c.append(xkv_1)
                for c in range(QC):
                    xq_1 = wqp.tile([128, IT, 512], BF16, name=f"xq{c}")
                    nc.gpsimd.dma_start(out=xq_1, in_=xqt[c, :, :, :])
                    xq_c.append(xq_1)
                def proj_k(ot):
                    for c in range(QC):
                        ps_k = psAO.tile([128, 512], F32, tag="psAO", name=f"ps_k{ot}_{c}")
                        for it in range(IT):
                            nc.tensor.matmul(
                                ps_k,
                                wk_sb[:, it, ot * 128 : (ot + 1) * 128],
                                xkv_c[c][:, it, :],
                                start=(it == 0),
                                stop=(it == IT - 1),
                            )
                        nc.vector.tensor_copy(kt_chunks[ot][c], ps_k)

                def proj_q(ot):
                    for c in range(QC):
                        ps_q = psAO.tile([128, 512], F32, tag="psAO", name=f"ps_q{ot}_{c}")
                        for it in range(IT):
                            nc.tensor.matmul(
                                ps_q,
                                wq_sb[:, it, ot * 128 : (ot + 1) * 128],
                                xq_c[c][:, it, :],
                                start=(it == 0),
                                stop=(it == IT - 1),
                            )
                        nc.vector.tensor_copy(qt_chunks[ot][c], ps_q)

                def proj_v(vt):
                    ps_v = psAO.tile([128, 512], F32, tag="psAO", name=f"ps_v{vt}")
                    for it in range(IT):
                        nc.tensor.matmul(
                            ps_v,
                            xkv_c[vt // 4][:, it, (vt % 4) * 128 : (vt % 4 + 1) * 128],
                            wv_sb[:, it, :],
                            start=(it == 0),
                            stop=(it == IT - 1),
                        )
                    nc.vector.tensor_copy(v_tiles[vt], ps_v)

                for vt in range(VT):
                    proj_v(vt)
                for ot in range(NP):
                    proj_k(ot)
                    proj_q(ot)

                # ---- Phase B: attention (pair-outer); Phase C: output proj ----
                ctx_tiles = [
                    ctxp.tile([128, NP, 512], BF16, name=f"ctx{c}") for c in range(QC)
                ]
                for qc in range(QC):
                    for j in range(NP):
                        psum_ctx = psC.tile([128, 512], F32, tag="psC")
                        psum_r = psR.tile([128, 512], F32, tag="psR")
                        for qi in range(4):  # quarters of the k range
                            pq = ppp.tile([128, 4, 2, 512], BF16, tag="pp")
                            for ki in range(4):  # per-ktile ACT batches
                                kt = qi * 4 + ki
                                ps_s = psS.tile([128, 2, 512], F32, tag="psS")
                                with tc.high_priority():
                                    nc.tensor.matmul(
                                        ps_s[:, 0, :],
                                        kt_chunks[j][kt // 4][0:64, (kt % 4) * 128 : (kt % 4 + 1) * 128],
                                        qt_chunks[j][qc][0:64, :],
                                        start=True,
                                        stop=True,
                                        tile_position=(0, 0),
                                    )
                                    nc.tensor.matmul(
                                        ps_s[:, 1, :],
                                        kt_chunks[j][kt // 4][64:128, (kt % 4) * 128 : (kt % 4 + 1) * 128],
                                        qt_chunks[j][qc][64:128, :],
                                        start=True,
                                        stop=True,
                                        tile_position=(64, 0),
                                    )
                                    nc.scalar.activation(
                                        out=pq[:, ki, :, :],
                                        in_=ps_s[:, :, :],
                                        func=mybir.ActivationFunctionType.Exp,
                                        scale=SCALE,
                                    )
                            # PV + sums for this quarter
                            with tc.high_priority():
                                for ki in range(4):
                                    kt = qi * 4 + ki
                                    first = kt == 0
                                    last = kt == KT - 1
                                    nc.tensor.matmul(
                                        psum_ctx[0:64, :],
                                        v_tiles[kt][:, j * 128 : j * 128 + 64],
                                        pq[:, ki, 0, :],
                                        start=first,
                                        stop=last,
                                        tile_position=(0, 0),
                                    )
                                    nc.tensor.matmul(
                                        psum_ctx[64:128, :],
                                        v_tiles[kt][:, j * 128 + 64 : (j + 1) * 128],
                                        pq[:, ki, 1, :],
                                        start=first,
                                        stop=last,
                                        tile_position=(0, 64),
                                    )
                                tq = sump.tile([128, 2, 2, 512], BF16, tag="tq")
                                s1 = sump.tile([128, 2, 512], BF16, tag="s1")
                                with nc.allow_low_precision(reason="softmax sum partials"):
                                    nc.vector.tensor_add(
                                        tq, pq[:, 0:2, :, :], pq[:, 2:4, :, :]
                                    )
                                    nc.vector.tensor_add(
                                        s1, tq[:, 0, :, :], tq[:, 1, :, :]
                                    )
                                nc.tensor.matmul(
                                    psum_r[0:64, :],
                                    ones_bf,
                                    s1[:, 0, :],
                                    start=(qi == 0),
                                    stop=(qi == 3),
                                    tile_position=(0, 0),
                                )
                                nc.tensor.matmul(
                                    psum_r[64:128, :],
                                    ones_bf,
                                    s1[:, 1, :],
                                    start=(qi == 0),
                                    stop=(qi == 3),
                                    tile_position=(0, 64),
                                )
                        # normalize
                        with tc.high_priority():
                            r_tile = rp.tile([128, 512], F32, tag="r")
                            nc.vector.reciprocal_approx_fast(out=r_tile, in_=psum_r)
                            with nc.allow_low_precision(reason="bf16 ctx for PE"):
                                nc.vector.tensor_mul(
                                    ctx_tiles[qc][:, j, :], psum_ctx, r_tile
                                )

                # Phase C: output projection per q-chunk
                for qc in range(QC):
                    qsl = slice(qc * 512, (qc + 1) * 512)
                    for mt in range(MT):
                        ps_o = psAO.tile([128, 512], F32, tag="psAO", name=f"ps_o{qc}_{mt}")
                        for jt in range(NP):
                            nc.tensor.matmul(
                                ps_o,
                                wo_sb[:, jt, mt * 128 : (mt + 1) * 128],
                                ctx_tiles[qc][:, jt, :],
                                start=(jt == 0),
                                stop=(jt == NP - 1),
                            )
                        ot_sb = ots.tile([128, 512], F32, tag="ot")
                        nc.vector.tensor_copy(ot_sb, ps_o)
                        nc.sync.dma_start(
                            out=otp[mt * 128 : (mt + 1) * 128, qsl], in_=ot_sb
                        )
    nc.compile()
    return nc


def make_in_maps(query_input, kv_input, W_q, W_k, W_v, W_o):
    import ml_dtypes

    bf16 = ml_dtypes.bfloat16
    q = np.asarray(query_input, dtype=np.float32).astype(bf16)
    kv = np.asarray(kv_input, dtype=np.float32).astype(bf16)
    W_q = np.asarray(W_q, dtype=np.float32).astype(bf16)
    W_k = np.asarray(W_k, dtype=np.float32).astype(bf16)
    W_v = np.asarray(W_v, dtype=np.float32).astype(bf16)
    W_o = np.asarray(W_o, dtype=np.float32).astype(bf16)
    ones = np.ones((128, 64), dtype=bf16)
    def tile_x(xt):  # [D, S] -> [QC, 128, IT, 512]
        return np.ascontiguousarray(
            xt.reshape(IT, 128, QC, 512).transpose(2, 1, 0, 3)
        )

    def tile_w(wt):  # [D, O] -> [128, IT, O]
        return np.ascontiguousarray(wt.reshape(IT, 128, O).transpose(1, 0, 2))

    in_maps = []
    for c in range(8):
        b, g = c // 2, c % 2
        sl = slice(g * O, (g + 1) * O)
        in_maps.append(
            {
                "xqt": tile_x(q[b].T),
                "xkvt": tile_x(kv[b].T),
                "wqt": tile_w(W_q[sl, :].T),
                "wkt": tile_w(W_k[sl, :].T),
                "wvt": tile_w(W_v[sl, :].T),
                "wot": np.ascontiguousarray(
                    W_o[:, sl].T.reshape(O // 128, 128, D).transpose(1, 0, 2)
                ),
                "ones": ones,
            }
        )
    return in_maps


def assemble_output(results):
    out = np.empty((4, S, D), dtype=np.float32)
    for b in range(4):
        partial = results[2 * b]["otp"] + results[2 * b + 1]["otp"]  # [D, S]
        out[b] = partial.T
    return out


_NC_CACHE = None


def kernel(**inputs) -> np.ndarray:
    global _NC_CACHE
    from concourse.bass_utils import run_bass_kernel_spmd

    if _NC_CACHE is None:
        _NC_CACHE = build_kernel()
    in_maps = make_in_maps(
        inputs["query_input"],
        inputs["kv_input"],
        inputs["W_q"],
        inputs["W_k"],
        inputs["W_v"],
        inputs["W_o"],
    )
    res = run_bass_kernel_spmd(_NC_CACHE, in_maps, list(range(8)))
    return assemble_output(res.results)
